# revision 1
# baseline (speedup 1.0000x reference)
"""Depth-warping layer for Trainium2 (Bass/Tile), 8-core data-parallel.

Strategy
--------
Pure data parallelism over batch: each of 8 NeuronCores handles 2 of 16
images. Per image:

Phase A: build a quad table J2[(H+1) x (W+1), 4] in device DRAM where
  J2[r, c] = (I[r-1,c-1], I[r,c-1], I[r-1,c], I[r,c]) for r in 1..H-1,
  c in 1..W-1, and rows 0/H plus cols 0/W are ZERO.  I = d1_calc.
  Zero borders exploit that the reference's clip-gather yields exactly 0
  whenever the warp coordinate leaves [0,W-1)x[0,H-1), so neither the
  weights nor the taps need clipping -- only the table index does.
  The quad interleave is built in SBUF (ACT-engine strided writes with
  per-partition bias) and stored with ONE contiguous DMA per 128-row
  block, instead of 4-byte-strided DRAM writes.

Phase B (hardware For_i loop over 8 row tiles): streaming coordinate
  math split across DVE and ACT (validated bit-faithful against the
  reference: magic-number floor, Relu-based index clip, float flat-index
  with exact integer arithmetic, HW reciprocal is correctly rounded),
  then 1280 per-partition-single-offset indirect DMA gathers (128
  descriptors each; the only reliable data-dependent addressing
  primitive on this stack -- measured 1.43us/instr, fully Pool-DGE
  serialized, which is the kernel's floor), then the 4-tap combine.

Host does only the O(1) 3x3 matrix algebra per batch and ships
per-batch / per-tile coefficients as small tensors.
"""

import numpy as np

import concourse.bass as bass
import concourse.bacc as bacc
import concourse.mybir as mybir
from concourse.tile import TileContext
from concourse import bass_utils

B, H, W = 16, 1024, 1280
NCORES = 8
BPC = B // NCORES
HP = H + 1
WP = W + 1
NTILES = H // 128

F32 = mybir.dt.float32
I32 = mybir.dt.int32
OP = mybir.AluOpType
AF = mybir.ActivationFunctionType

MAGIC = 12582912.0            # 1.5 * 2**23
C0 = float(H * WP + W)        # 1313024: flat index of the zeroed corner cell


def _build_bass():
    nc = bacc.Bacc(target_bir_lowering=False, num_swdge_queues=4)

    d1 = nc.dram_tensor("d1", [BPC, H, W], F32, kind="ExternalInput")
    d2 = nc.dram_tensor("d2", [BPC, H, W], F32, kind="ExternalInput")
    rowA = nc.dram_tensor("rowA", [BPC, 128, W], F32, kind="ExternalInput")   # M00*x
    rowB = nc.dram_tensor("rowB", [BPC, 128, W], F32, kind="ExternalInput")   # M10*x
    rowC = nc.dram_tensor("rowC", [BPC, 128, W], F32, kind="ExternalInput")   # M20*x
    rowG = nc.dram_tensor("rowG", [BPC, 128, W], F32, kind="ExternalInput")   # M2_20*x
    # per-tile biases, sliced by the For_i loop var: [BPC, NTILES, 128, 1]
    cA = nc.dram_tensor("cA", [BPC, NTILES, 128, 1], F32, kind="ExternalInput")  # M01*y+M02
    cB = nc.dram_tensor("cB", [BPC, NTILES, 128, 1], F32, kind="ExternalInput")  # M11*y+M12
    cC = nc.dram_tensor("cC", [BPC, NTILES, 128, 1], F32, kind="ExternalInput")  # M21*y+M22
    # phase-A biases (python-unrolled): [BPC, 128, NTILES]
    gC = nc.dram_tensor("gC", [BPC, 128, NTILES], F32, kind="ExternalInput")  # M2_21*(r)+M2_22, cur rows
    gP = nc.dram_tensor("gP", [BPC, 128, NTILES], F32, kind="ExternalInput")  # prev rows
    wx = nc.dram_tensor("wx", [BPC, 128, 1], F32, kind="ExternalInput")  # Wv0
    wy = nc.dram_tensor("wy", [BPC, 128, 1], F32, kind="ExternalInput")  # Wv1
    wz = nc.dram_tensor("wz", [BPC, 128, 1], F32, kind="ExternalInput")  # Wv2
    w2 = nc.dram_tensor("w2", [BPC, 128, 1], F32, kind="ExternalInput")  # W2z
    out = nc.dram_tensor("out", [BPC, H, W], F32, kind="ExternalOutput")

    with TileContext(nc) as tc:
        with tc.tile_pool(name="dram", bufs=2, space="DRAM") as dpool, \
             tc.tile_pool(name="cst", bufs=1) as cpool, \
             tc.tile_pool(name="sc", bufs=1) as sp, \
             tc.tile_pool(name="qa", bufs=1) as qa, \
             tc.tile_pool(name="io", bufs=2) as iop:

            zrow = cpool.tile([128, 40], F32)
            nc.vector.memset(zrow[:], 0.0)
            c1 = cpool.tile([128, 1], F32)
            cW = cpool.tile([128, 1], F32)
            cH = cpool.tile([128, 1], F32)
            nc.vector.memset(c1[:], 1.0)
            nc.vector.memset(cW[:], float(W))
            nc.vector.memset(cH[:], float(H))

            J2flats = []
            for lb in range(BPC):
                # ---- Phase-A constants (bufs=2 so batch 1 overlaps batch 0's gathers) ----
                rowG_t = cpool.tile([128, W], F32, tag="rowG", bufs=2)
                nc.sync.dma_start(out=rowG_t[:], in_=rowG[lb])
                gC_t = cpool.tile([128, NTILES], F32, tag="gC", bufs=2)
                gP_t = cpool.tile([128, NTILES], F32, tag="gP", bufs=2)
                nc.sync.dma_start(out=gC_t[:], in_=gC[lb])
                nc.sync.dma_start(out=gP_t[:], in_=gP[lb])
                w2_t = cpool.tile([128, 1], F32, tag="w2", bufs=2)
                nc.sync.dma_start(out=w2_t[:], in_=w2[lb])

                J2 = dpool.tile([HP, WP, 4], F32, tag="J2")
                J2flats.append(J2[:].rearrange("a b c -> (a b) c"))

                # ---- Phase A ----
                nc.sync.dma_start(out=J2[0:1, 0:W, :], in_=zrow[:, :])
                nc.sync.dma_start(out=J2[0:1, W, :], in_=zrow[0:1, 0:4])
                for b in range(NTILES):
                    r0 = 1 + 128 * b
                    d2c = sp.tile([128, W], F32, tag="d2c")
                    d2p = sp.tile([128, W], F32, tag="d2p")
                    if b < NTILES - 1:
                        nc.sync.dma_start(out=d2c[:], in_=d2[lb, r0:r0 + 128, :])
                    else:
                        nc.sync.dma_start(out=d2c[0:127, :], in_=d2[lb, r0:H, :])
                        nc.sync.dma_start(out=d2c[127:128, :], in_=d2[lb, H - 1:H, :])
                    nc.sync.dma_start(out=d2p[:], in_=d2[lb, r0 - 1:r0 + 127, :])
                    Ic = sp.tile([128, W], F32, tag="pIc")
                    Ip = sp.tile([128, W], F32, tag="pIp")
                    nc.scalar.activation(out=Ic[:], in_=rowG_t[:], func=AF.Identity,
                                         bias=gC_t[:, b:b + 1])
                    nc.scalar.activation(out=Ip[:], in_=rowG_t[:], func=AF.Identity,
                                         bias=gP_t[:, b:b + 1])
                    nc.vector.tensor_tensor(out=Ic[:], in0=Ic[:], in1=d2c[:], op=OP.mult)
                    nc.vector.tensor_tensor(out=Ip[:], in0=Ip[:], in1=d2p[:], op=OP.mult)
                    Q = qa.tile([128, WP, 4], F32, tag="Q")
                    nc.vector.memset(Q[:, 0, :], 0.0)
                    nc.vector.memset(Q[:, W, :], 0.0)
                    nc.scalar.activation(out=Q[:, 1:W, 0], in_=Ip[:, 0:W - 1],
                                         func=AF.Identity, bias=w2_t[:, 0:1])
                    nc.scalar.activation(out=Q[:, 1:W, 1], in_=Ic[:, 0:W - 1],
                                         func=AF.Identity, bias=w2_t[:, 0:1])
                    nc.scalar.activation(out=Q[:, 1:W, 2], in_=Ip[:, 1:W],
                                         func=AF.Identity, bias=w2_t[:, 0:1])
                    nc.scalar.activation(out=Q[:, 1:W, 3], in_=Ic[:, 1:W],
                                         func=AF.Identity, bias=w2_t[:, 0:1])
                    nc.sync.dma_start(out=J2[r0:r0 + 128, :, :], in_=Q[:])
                nc.sync.dma_start(out=J2[H:H + 1, 0:W, :], in_=zrow[:, :])
                nc.sync.dma_start(out=J2[H:H + 1, W, :], in_=zrow[0:1, 0:4])

            for lb in range(BPC):
                # ---- Phase-B constants ----
                rowA_t = cpool.tile([128, W], F32, tag="rowA")
                rowB_t = cpool.tile([128, W], F32, tag="rowB")
                rowC_t = cpool.tile([128, W], F32, tag="rowC")
                nc.sync.dma_start(out=rowA_t[:], in_=rowA[lb])
                nc.sync.dma_start(out=rowB_t[:], in_=rowB[lb])
                nc.sync.dma_start(out=rowC_t[:], in_=rowC[lb])
                wx_t = cpool.tile([128, 1], F32, tag="wx")
                wy_t = cpool.tile([128, 1], F32, tag="wy")
                wz_t = cpool.tile([128, 1], F32, tag="wz")
                nc.sync.dma_start(out=wx_t[:], in_=wx[lb])
                nc.sync.dma_start(out=wy_t[:], in_=wy[lb])
                nc.sync.dma_start(out=wz_t[:], in_=wz[lb])
                J2flat = J2flats[lb]

                # ---- Phase B: 2-pipe For_i body ----
                d1v = d1[lb].rearrange("(s q p) w -> s q p w", q=2, p=128)
                outv = out[lb].rearrange("(s q p) w -> s q p w", q=2, p=128)
                cAv = cA[lb].rearrange("(s q) p o -> s q p o", q=2)
                cBv = cB[lb].rearrange("(s q) p o -> s q p o", q=2)
                cCv = cC[lb].rearrange("(s q) p o -> s q p o", q=2)

                with tc.For_i(0, NTILES // 2, 1) as sv:
                    pipes = []
                    for pi in (0, 1):
                        z1 = iop.tile([128, W], F32, tag=f"z1{pi}", bufs=1)
                        nc.sync.dma_start(out=z1[:], in_=d1v[bass.ds(sv, 1), pi, :, :])
                        bA = sp.tile([128, 1], F32, tag=f"bA{pi}")
                        bB = sp.tile([128, 1], F32, tag=f"bB{pi}")
                        bC = sp.tile([128, 1], F32, tag=f"bC{pi}")
                        nc.sync.dma_start(out=bA[:], in_=cAv[bass.ds(sv, 1), pi, :, :])
                        nc.sync.dma_start(out=bB[:], in_=cBv[bass.ds(sv, 1), pi, :, :])
                        nc.sync.dma_start(out=bC[:], in_=cCv[bass.ds(sv, 1), pi, :, :])

                        A = sp.tile([128, W], F32, tag="sA")
                        Bt = sp.tile([128, W], F32, tag="sB")
                        Ct = sp.tile([128, W], F32, tag="sC")
                        nc.scalar.activation(out=A[:], in_=rowA_t[:], func=AF.Identity,
                                             bias=bA[:, 0:1])
                        nc.scalar.activation(out=Bt[:], in_=rowB_t[:], func=AF.Identity,
                                             bias=bB[:, 0:1])
                        nc.scalar.activation(out=Ct[:], in_=rowC_t[:], func=AF.Identity,
                                             bias=bC[:, 0:1])
                        z2 = sp.tile([128, W], F32, tag="sD")
                        nc.vector.tensor_tensor(out=z2[:], in0=z1[:], in1=Ct[:], op=OP.mult)
                        nc.scalar.activation(out=z2[:], in_=z2[:], func=AF.Identity,
                                             bias=wz_t[:, 0:1])
                        r = sp.tile([128, W], F32, tag="sE")
                        nc.vector.reciprocal(out=r[:], in_=z2[:])
                        nU = sp.tile([128, W], F32, tag="sF")
                        nV = sp.tile([128, W], F32, tag="sG")
                        nc.vector.tensor_tensor(out=nU[:], in0=z1[:], in1=A[:], op=OP.mult)
                        nc.scalar.activation(out=nU[:], in_=nU[:], func=AF.Identity,
                                             bias=wx_t[:, 0:1])
                        nc.vector.tensor_tensor(out=nV[:], in0=z1[:], in1=Bt[:], op=OP.mult)
                        nc.scalar.activation(out=nV[:], in_=nV[:], func=AF.Identity,
                                             bias=wy_t[:, 0:1])
                        u2 = sp.tile([128, W], F32, tag="sH")
                        v2 = sp.tile([128, W], F32, tag="sI")
                        nc.vector.tensor_tensor(out=u2[:], in0=nU[:], in1=r[:], op=OP.mult)
                        nc.vector.tensor_tensor(out=v2[:], in0=nV[:], in1=r[:], op=OP.mult)
                        x0f = sp.tile([128, W], F32, tag="sF")
                        y0f = sp.tile([128, W], F32, tag="sG")
                        nc.scalar.activation(out=x0f[:], in_=u2[:], func=AF.Copy, bias=-0.5)
                        nc.scalar.activation(out=x0f[:], in_=x0f[:], func=AF.Copy, bias=MAGIC)
                        nc.scalar.activation(out=x0f[:], in_=x0f[:], func=AF.Copy, bias=-MAGIC)
                        nc.scalar.activation(out=y0f[:], in_=v2[:], func=AF.Copy, bias=-0.5)
                        nc.scalar.activation(out=y0f[:], in_=y0f[:], func=AF.Copy, bias=MAGIC)
                        nc.scalar.activation(out=y0f[:], in_=y0f[:], func=AF.Copy, bias=-MAGIC)
                        frx = sp.tile([128, W], F32, tag=f"frx{pi}")
                        fry = sp.tile([128, W], F32, tag=f"fry{pi}")
                        nc.vector.tensor_tensor(out=frx[:], in0=u2[:], in1=x0f[:], op=OP.subtract)
                        nc.vector.tensor_tensor(out=fry[:], in0=v2[:], in1=y0f[:], op=OP.subtract)
                        rx = sp.tile([128, W], F32, tag="sA")
                        ry = sp.tile([128, W], F32, tag="sB")
                        nc.scalar.activation(out=rx[:], in_=x0f[:], func=AF.Relu, bias=c1[:, 0:1])
                        nc.scalar.activation(out=rx[:], in_=rx[:], func=AF.Relu,
                                             scale=-1.0, bias=cW[:, 0:1])
                        nc.scalar.activation(out=ry[:], in_=y0f[:], func=AF.Relu, bias=c1[:, 0:1])
                        nc.scalar.activation(out=ry[:], in_=ry[:], func=AF.Relu,
                                             scale=-1.0, bias=cH[:, 0:1])
                        nc.vector.scalar_tensor_tensor(out=rx[:], in0=ry[:], scalar=float(WP),
                                                       in1=rx[:], op0=OP.mult, op1=OP.add)
                        nc.scalar.activation(out=rx[:], in_=rx[:], func=AF.Copy,
                                             scale=-1.0, bias=C0)
                        flat = sp.tile([128, W], I32, tag=f"flat{pi}")
                        nc.vector.tensor_copy(out=flat[:], in_=rx[:])

                        gq = sp.tile([128, W, 4], F32, tag=f"gq{pi}")
                        for j in range(W):
                            inst = nc.gpsimd.indirect_dma_start(
                                out=gq[:, j, :], out_offset=None,
                                in_=J2flat,
                                in_offset=bass.IndirectOffsetOnAxis(ap=flat[:, j:j + 1], axis=0),
                            )
                            inst.ins.queue = f"qPoolDynamic{j % 4 or ''}"
                        pipes.append((pi, frx, fry, gq))

                    for pi, frx, fry, gq in pipes:
                        # bilinear as two lerps: s = q_lo + fr*(q_hi - q_lo)
                        t1 = sp.tile([128, W], F32, tag="sD")
                        t2 = sp.tile([128, W], F32, tag="sE")
                        ot = iop.tile([128, W], F32, tag=f"ot{pi}", bufs=1)
                        nc.vector.tensor_tensor(out=t1[:], in0=gq[:, :, 2], in1=gq[:, :, 0], op=OP.subtract)
                        nc.vector.tensor_tensor(out=t1[:], in0=frx[:], in1=t1[:], op=OP.mult)
                        nc.vector.tensor_tensor(out=t1[:], in0=gq[:, :, 0], in1=t1[:], op=OP.add)
                        nc.vector.tensor_tensor(out=t2[:], in0=gq[:, :, 3], in1=gq[:, :, 1], op=OP.subtract)
                        nc.vector.tensor_tensor(out=t2[:], in0=frx[:], in1=t2[:], op=OP.mult)
                        nc.vector.tensor_tensor(out=t2[:], in0=gq[:, :, 1], in1=t2[:], op=OP.add)
                        nc.vector.tensor_tensor(out=t2[:], in0=t2[:], in1=t1[:], op=OP.subtract)
                        nc.vector.tensor_tensor(out=t2[:], in0=fry[:], in1=t2[:], op=OP.mult)
                        nc.vector.tensor_tensor(out=ot[:], in0=t1[:], in1=t2[:], op=OP.add)
                        nc.sync.dma_start(out=outv[bass.ds(sv, 1), pi, :, :], in_=ot[:])

    nc.finalize()
    return nc


def _host_aux(translation, rotation, intrinsic):
    K = intrinsic.astype(np.float32)
    Kinv = np.linalg.inv(K).astype(np.float32)
    R = rotation.astype(np.float32)
    t = translation.astype(np.float32)
    nb = R.shape[0]
    temp = np.einsum('ij,bkj->bik', K, R).astype(np.float32)
    Wv = np.einsum('bij,bjk->bik', temp, -t).astype(np.float32)
    M = np.einsum('bij,jk->bik', temp, Kinv).astype(np.float32)
    W2 = np.einsum('ij,bjk->bik', K, t).astype(np.float32)
    M2 = np.einsum('bij,jk->bik', np.einsum('ij,bjk->bik', K, R), Kinv).astype(np.float32)

    x = np.arange(W, dtype=np.float32)
    y = np.arange(H, dtype=np.float32)
    ycols = y.reshape(NTILES, 128)                                   # [NTILES, 128]
    ycolsT = ycols.T                                                 # [128, NTILES]

    def rep_row(v):
        return np.repeat(v[:, None, :], 128, axis=1).astype(np.float32)

    aux = {}
    aux["rowA"] = rep_row(M[:, 0, 0][:, None] * x[None, :])
    aux["rowB"] = rep_row(M[:, 1, 0][:, None] * x[None, :])
    aux["rowC"] = rep_row(M[:, 2, 0][:, None] * x[None, :])
    aux["rowG"] = rep_row(M2[:, 2, 0][:, None] * x[None, :])
    # [nb, NTILES, 128, 1]
    aux["cA"] = (M[:, 0, 1][:, None, None] * ycols[None] + M[:, 0, 2][:, None, None]).astype(np.float32)[..., None]
    aux["cB"] = (M[:, 1, 1][:, None, None] * ycols[None] + M[:, 1, 2][:, None, None]).astype(np.float32)[..., None]
    aux["cC"] = (M[:, 2, 1][:, None, None] * ycols[None] + M[:, 2, 2][:, None, None]).astype(np.float32)[..., None]
    # phase A: cur rows r = 1+128b+p; prev rows r-1 = 128b+p  -> [nb, 128, NTILES]
    aux["gC"] = (M2[:, 2, 1][:, None, None] * (ycolsT[None] + 1.0) + M2[:, 2, 2][:, None, None]).astype(np.float32)
    aux["gP"] = (M2[:, 2, 1][:, None, None] * ycolsT[None] + M2[:, 2, 2][:, None, None]).astype(np.float32)
    ones = np.ones((nb, 128, 1), np.float32)
    aux["wx"] = Wv[:, 0, 0][:, None, None] * ones
    aux["wy"] = Wv[:, 1, 0][:, None, None] * ones
    aux["wz"] = Wv[:, 2, 0][:, None, None] * ones
    aux["w2"] = W2[:, 2, 0][:, None, None] * ones
    for k in aux:
        aux[k] = np.ascontiguousarray(aux[k].astype(np.float32))
    return aux


_NC_CACHE = {}


def kernel(depth_map_1, depth_map_2, translation, rotation, intrinsic):
    d1 = np.ascontiguousarray(np.asarray(depth_map_1, dtype=np.float32)[..., 0])
    d2 = np.ascontiguousarray(np.asarray(depth_map_2, dtype=np.float32)[..., 0])
    t = np.asarray(translation, dtype=np.float32)
    R = np.asarray(rotation, dtype=np.float32)
    K = np.asarray(intrinsic, dtype=np.float32)

    if "nc" not in _NC_CACHE:
        _NC_CACHE["nc"] = _build_bass()
    nc = _NC_CACHE["nc"]

    aux = _host_aux(t, R, K)

    in_maps = []
    for c in range(NCORES):
        sl = slice(c * BPC, (c + 1) * BPC)
        m = {"d1": d1[sl], "d2": d2[sl]}
        for k, v in aux.items():
            m[k] = v[sl]
        in_maps.append(m)

    res = bass_utils.run_bass_kernel_spmd(nc, in_maps, core_ids=list(range(NCORES)))
    outa = np.empty((B, H, W, 1), np.float32)
    for c in range(NCORES):
        outa[c * BPC:(c + 1) * BPC, :, :, 0] = res.results[c]["out"]
    return outa



# revision 3
# speedup vs baseline: 1.0041x; 1.0041x over previous
"""Depth-warping layer for Trainium2 (Bass/Tile), 8-core data-parallel.

Strategy
--------
Pure data parallelism over batch: each of 8 NeuronCores handles 2 of 16
images. Per image:

Phase A: build a PAIR table P[(H+1) x (W+2), 2] in device DRAM where
  P[r, c] = (I[r-1, c-1], I[r, c-1]) for r in 1..H-1, c in 1..W-1, and
  rows 0/H plus cols 0/W/W+1 are ZERO.  I = d1_calc (with the +W2z bias
  baked into every cell so zero border cells stay exactly zero).
  Reading 4 consecutive floats at cell (r, c) yields
  (I[r-1,c-1], I[r,c-1], I[r-1,c], I[r,c]) - the same bilinear tap quad
  the previous quad-table layout stored explicitly, at HALF the table
  bytes (10.5MB vs 21MB per image) and half the strided interleave
  writes in Phase A.  The extra zero column W+1 absorbs the 2-entry
  read at the clipped cell c == W so it never wraps into row r+1.
  Zero borders exploit that the reference's clip-gather yields exactly 0
  whenever the warp coordinate leaves [0,W-1)x[0,H-1), so neither the
  weights nor the taps need clipping -- only the table index does.

Phase B (hardware For_i loop over 8 row tiles): streaming coordinate
  math split across DVE and ACT (validated bit-faithful against the
  reference: magic-number floor, Relu-based index clip, float flat-index
  with exact integer arithmetic, HW reciprocal is correctly rounded),
  then 1280 per-partition-single-offset indirect DMA gathers (128
  descriptors each, 4 floats per descriptor; the only reliable
  data-dependent addressing primitive on this stack -- measured
  1.41us/instr back-to-back with zero deps, fully Pool-DGE serialized,
  which is the kernel's floor), then the 4-tap combine.

Host does only the O(1) 3x3 matrix algebra per batch and ships
per-batch / per-tile coefficients as small tensors.
"""

import numpy as np

import concourse.bass as bass
import concourse.bacc as bacc
import concourse.mybir as mybir
from concourse.tile import TileContext
from concourse import bass_utils

B, H, W = 16, 1024, 1280
NCORES = 8
BPC = B // NCORES
HP = H + 1
WP2 = W + 2
NTILES = H // 128

F32 = mybir.dt.float32
I32 = mybir.dt.int32
OP = mybir.AluOpType
AF = mybir.ActivationFunctionType

MAGIC = 12582912.0            # 1.5 * 2**23
C0 = float(H * WP2 + W)       # flat index of the zeroed far corner cell


def _build_bass():
    nc = bacc.Bacc(target_bir_lowering=False, num_swdge_queues=4)

    d1 = nc.dram_tensor("d1", [BPC, H, W], F32, kind="ExternalInput")
    d2 = nc.dram_tensor("d2", [BPC, H, W], F32, kind="ExternalInput")
    rowA = nc.dram_tensor("rowA", [BPC, 128, W], F32, kind="ExternalInput")   # M00*x
    rowB = nc.dram_tensor("rowB", [BPC, 128, W], F32, kind="ExternalInput")   # M10*x
    rowC = nc.dram_tensor("rowC", [BPC, 128, W], F32, kind="ExternalInput")   # M20*x
    rowG = nc.dram_tensor("rowG", [BPC, 128, W], F32, kind="ExternalInput")   # M2_20*x
    # per-tile biases, sliced by the For_i loop var: [BPC, NTILES, 128, 1]
    cA = nc.dram_tensor("cA", [BPC, NTILES, 128, 1], F32, kind="ExternalInput")  # M01*y+M02
    cB = nc.dram_tensor("cB", [BPC, NTILES, 128, 1], F32, kind="ExternalInput")  # M11*y+M12
    cC = nc.dram_tensor("cC", [BPC, NTILES, 128, 1], F32, kind="ExternalInput")  # M21*y+M22
    # phase-A biases (python-unrolled): [BPC, 128, NTILES]
    gC = nc.dram_tensor("gC", [BPC, 128, NTILES], F32, kind="ExternalInput")  # M2_21*(r)+M2_22, cur rows
    gP = nc.dram_tensor("gP", [BPC, 128, NTILES], F32, kind="ExternalInput")  # prev rows
    wx = nc.dram_tensor("wx", [BPC, 128, 1], F32, kind="ExternalInput")  # Wv0
    wy = nc.dram_tensor("wy", [BPC, 128, 1], F32, kind="ExternalInput")  # Wv1
    wz = nc.dram_tensor("wz", [BPC, 128, 1], F32, kind="ExternalInput")  # Wv2
    w2 = nc.dram_tensor("w2", [BPC, 128, 1], F32, kind="ExternalInput")  # W2z
    out = nc.dram_tensor("out", [BPC, H, W], F32, kind="ExternalOutput")

    with TileContext(nc) as tc:
        with tc.tile_pool(name="dram", bufs=2, space="DRAM") as dpool, \
             tc.tile_pool(name="cst", bufs=1) as cpool, \
             tc.tile_pool(name="sc", bufs=1) as sp, \
             tc.tile_pool(name="qa", bufs=1) as qa, \
             tc.tile_pool(name="io", bufs=2) as iop:

            zrow = cpool.tile([1, WP2 * 2], F32)
            nc.vector.memset(zrow[:], 0.0)
            c1 = cpool.tile([128, 1], F32)
            cW = cpool.tile([128, 1], F32)
            cH = cpool.tile([128, 1], F32)
            nc.vector.memset(c1[:], 1.0)
            nc.vector.memset(cW[:], float(W))
            nc.vector.memset(cH[:], float(H))

            J2flats = []
            for lb in range(BPC):
                # ---- Phase-A constants (bufs=2 so batch 1 overlaps batch 0's gathers) ----
                rowG_t = cpool.tile([128, W], F32, tag="rowG", bufs=2)
                nc.sync.dma_start(out=rowG_t[:], in_=rowG[lb])
                gC_t = cpool.tile([128, NTILES], F32, tag="gC", bufs=2)
                gP_t = cpool.tile([128, NTILES], F32, tag="gP", bufs=2)
                nc.sync.dma_start(out=gC_t[:], in_=gC[lb])
                nc.sync.dma_start(out=gP_t[:], in_=gP[lb])
                w2_t = cpool.tile([128, 1], F32, tag="w2", bufs=2)
                nc.sync.dma_start(out=w2_t[:], in_=w2[lb])

                J2 = dpool.tile([HP, WP2, 2], F32, tag="J2")
                J2flats.append(J2[:].rearrange("a b c -> (a b) c"))

                # ---- Phase A ----
                nc.sync.dma_start(out=J2[0:1, :, :].rearrange("a b c -> a (b c)"),
                                  in_=zrow[:, :])
                for b in range(NTILES):
                    r0 = 1 + 128 * b
                    d2c = sp.tile([128, W], F32, tag="d2c")
                    d2p = sp.tile([128, W], F32, tag="d2p")
                    if b < NTILES - 1:
                        nc.sync.dma_start(out=d2c[:], in_=d2[lb, r0:r0 + 128, :])
                    else:
                        nc.sync.dma_start(out=d2c[0:127, :], in_=d2[lb, r0:H, :])
                        nc.sync.dma_start(out=d2c[127:128, :], in_=d2[lb, H - 1:H, :])
                    nc.sync.dma_start(out=d2p[:], in_=d2[lb, r0 - 1:r0 + 127, :])
                    Ic = sp.tile([128, W], F32, tag="pIc")
                    Ip = sp.tile([128, W], F32, tag="pIp")
                    nc.scalar.activation(out=Ic[:], in_=rowG_t[:], func=AF.Identity,
                                         bias=gC_t[:, b:b + 1])
                    nc.scalar.activation(out=Ip[:], in_=rowG_t[:], func=AF.Identity,
                                         bias=gP_t[:, b:b + 1])
                    nc.vector.tensor_tensor(out=Ic[:], in0=Ic[:], in1=d2c[:], op=OP.mult)
                    nc.vector.tensor_tensor(out=Ip[:], in0=Ip[:], in1=d2p[:], op=OP.mult)
                    Q = qa.tile([128, WP2, 2], F32, tag="Q")
                    nc.vector.memset(Q[:, 0, :], 0.0)
                    nc.vector.memset(Q[:, W:W + 2, :], 0.0)
                    nc.scalar.activation(out=Q[:, 1:W, 0], in_=Ip[:, 0:W - 1],
                                         func=AF.Identity, bias=w2_t[:, 0:1])
                    nc.scalar.activation(out=Q[:, 1:W, 1], in_=Ic[:, 0:W - 1],
                                         func=AF.Identity, bias=w2_t[:, 0:1])
                    nc.sync.dma_start(out=J2[r0:r0 + 128, :, :], in_=Q[:])
                nc.sync.dma_start(out=J2[H:H + 1, :, :].rearrange("a b c -> a (b c)"),
                                  in_=zrow[:, :])

            for lb in range(BPC):
                # ---- Phase-B constants ----
                rowA_t = cpool.tile([128, W], F32, tag="rowA")
                rowB_t = cpool.tile([128, W], F32, tag="rowB")
                rowC_t = cpool.tile([128, W], F32, tag="rowC")
                nc.sync.dma_start(out=rowA_t[:], in_=rowA[lb])
                nc.sync.dma_start(out=rowB_t[:], in_=rowB[lb])
                nc.sync.dma_start(out=rowC_t[:], in_=rowC[lb])
                wx_t = cpool.tile([128, 1], F32, tag="wx")
                wy_t = cpool.tile([128, 1], F32, tag="wy")
                wz_t = cpool.tile([128, 1], F32, tag="wz")
                nc.sync.dma_start(out=wx_t[:], in_=wx[lb])
                nc.sync.dma_start(out=wy_t[:], in_=wy[lb])
                nc.sync.dma_start(out=wz_t[:], in_=wz[lb])
                J2flat = J2flats[lb]

                # ---- Phase B: 2-pipe For_i body ----
                d1v = d1[lb].rearrange("(s q p) w -> s q p w", q=2, p=128)
                outv = out[lb].rearrange("(s q p) w -> s q p w", q=2, p=128)
                cAv = cA[lb].rearrange("(s q) p o -> s q p o", q=2)
                cBv = cB[lb].rearrange("(s q) p o -> s q p o", q=2)
                cCv = cC[lb].rearrange("(s q) p o -> s q p o", q=2)

                with tc.For_i(0, NTILES // 2, 1) as sv:
                    pipes = []
                    for pi in (0, 1):
                        z1 = iop.tile([128, W], F32, tag=f"z1{pi}", bufs=1)
                        nc.sync.dma_start(out=z1[:], in_=d1v[bass.ds(sv, 1), pi, :, :])
                        bA = sp.tile([128, 1], F32, tag=f"bA{pi}")
                        bB = sp.tile([128, 1], F32, tag=f"bB{pi}")
                        bC = sp.tile([128, 1], F32, tag=f"bC{pi}")
                        nc.sync.dma_start(out=bA[:], in_=cAv[bass.ds(sv, 1), pi, :, :])
                        nc.sync.dma_start(out=bB[:], in_=cBv[bass.ds(sv, 1), pi, :, :])
                        nc.sync.dma_start(out=bC[:], in_=cCv[bass.ds(sv, 1), pi, :, :])

                        A = sp.tile([128, W], F32, tag="sA")
                        Bt = sp.tile([128, W], F32, tag="sB")
                        Ct = sp.tile([128, W], F32, tag="sC")
                        nc.scalar.activation(out=A[:], in_=rowA_t[:], func=AF.Identity,
                                             bias=bA[:, 0:1])
                        nc.scalar.activation(out=Bt[:], in_=rowB_t[:], func=AF.Identity,
                                             bias=bB[:, 0:1])
                        nc.scalar.activation(out=Ct[:], in_=rowC_t[:], func=AF.Identity,
                                             bias=bC[:, 0:1])
                        z2 = sp.tile([128, W], F32, tag="sD")
                        nc.vector.tensor_tensor(out=z2[:], in0=z1[:], in1=Ct[:], op=OP.mult)
                        nc.scalar.activation(out=z2[:], in_=z2[:], func=AF.Identity,
                                             bias=wz_t[:, 0:1])
                        r = sp.tile([128, W], F32, tag="sE")
                        nc.vector.reciprocal(out=r[:], in_=z2[:])
                        nU = sp.tile([128, W], F32, tag="sF")
                        nV = sp.tile([128, W], F32, tag="sG")
                        nc.vector.tensor_tensor(out=nU[:], in0=z1[:], in1=A[:], op=OP.mult)
                        nc.scalar.activation(out=nU[:], in_=nU[:], func=AF.Identity,
                                             bias=wx_t[:, 0:1])
                        nc.vector.tensor_tensor(out=nV[:], in0=z1[:], in1=Bt[:], op=OP.mult)
                        nc.scalar.activation(out=nV[:], in_=nV[:], func=AF.Identity,
                                             bias=wy_t[:, 0:1])
                        u2 = sp.tile([128, W], F32, tag="sH")
                        v2 = sp.tile([128, W], F32, tag="sI")
                        nc.vector.tensor_tensor(out=u2[:], in0=nU[:], in1=r[:], op=OP.mult)
                        nc.vector.tensor_tensor(out=v2[:], in0=nV[:], in1=r[:], op=OP.mult)
                        x0f = sp.tile([128, W], F32, tag="sF")
                        y0f = sp.tile([128, W], F32, tag="sG")
                        nc.scalar.activation(out=x0f[:], in_=u2[:], func=AF.Copy, bias=-0.5)
                        nc.scalar.activation(out=x0f[:], in_=x0f[:], func=AF.Copy, bias=MAGIC)
                        nc.scalar.activation(out=x0f[:], in_=x0f[:], func=AF.Copy, bias=-MAGIC)
                        nc.scalar.activation(out=y0f[:], in_=v2[:], func=AF.Copy, bias=-0.5)
                        nc.scalar.activation(out=y0f[:], in_=y0f[:], func=AF.Copy, bias=MAGIC)
                        nc.scalar.activation(out=y0f[:], in_=y0f[:], func=AF.Copy, bias=-MAGIC)
                        frx = sp.tile([128, W], F32, tag=f"frx{pi}")
                        fry = sp.tile([128, W], F32, tag=f"fry{pi}")
                        nc.vector.tensor_tensor(out=frx[:], in0=u2[:], in1=x0f[:], op=OP.subtract)
                        nc.vector.tensor_tensor(out=fry[:], in0=v2[:], in1=y0f[:], op=OP.subtract)
                        # rx1 = Relu(x0+1); m = min(rx1,1) flags "x0 >= 0"; left-outside
                        # pixels (m=0) are routed to the all-zero row 0 so the pair
                        # table's overlapping 2-cell read never touches real column-0
                        # data: flat = W - rx2 - WP2*m*(ry2 - H)
                        rx = sp.tile([128, W], F32, tag="sA")
                        ry = sp.tile([128, W], F32, tag="sB")
                        mm = sp.tile([128, W], F32, tag="sJ")
                        nc.scalar.activation(out=rx[:], in_=x0f[:], func=AF.Relu, bias=c1[:, 0:1])
                        nc.vector.tensor_scalar(out=mm[:], in0=rx[:], scalar1=1.0,
                                                scalar2=None, op0=OP.min)
                        nc.scalar.activation(out=rx[:], in_=rx[:], func=AF.Relu,
                                             scale=-1.0, bias=cW[:, 0:1])
                        nc.scalar.activation(out=ry[:], in_=y0f[:], func=AF.Relu, bias=c1[:, 0:1])
                        nc.scalar.activation(out=ry[:], in_=ry[:], func=AF.Relu,
                                             scale=-1.0, bias=cH[:, 0:1])
                        nc.scalar.activation(out=ry[:], in_=ry[:], func=AF.Copy,
                                             bias=-float(H))
                        nc.vector.tensor_tensor(out=mm[:], in0=mm[:], in1=ry[:], op=OP.mult)
                        nc.vector.scalar_tensor_tensor(out=rx[:], in0=mm[:], scalar=float(WP2),
                                                       in1=rx[:], op0=OP.mult, op1=OP.add)
                        nc.scalar.activation(out=rx[:], in_=rx[:], func=AF.Copy,
                                             scale=-1.0, bias=float(W))
                        flat = sp.tile([128, W], I32, tag=f"flat{pi}")
                        nc.vector.tensor_copy(out=flat[:], in_=rx[:])

                        gq = sp.tile([128, W, 4], F32, tag=f"gq{pi}")
                        for j in range(W):
                            inst = nc.gpsimd.indirect_dma_start(
                                out=gq[:, j, :], out_offset=None,
                                in_=J2flat,
                                in_offset=bass.IndirectOffsetOnAxis(ap=flat[:, j:j + 1], axis=0),
                            )
                            inst.ins.queue = f"qPoolDynamic{j % 4 or ''}"
                        pipes.append((pi, frx, fry, gq))

                    for pi, frx, fry, gq in pipes:
                        # bilinear as two lerps: s = q_lo + fr*(q_hi - q_lo)
                        t1 = sp.tile([128, W], F32, tag="sD")
                        t2 = sp.tile([128, W], F32, tag="sE")
                        ot = iop.tile([128, W], F32, tag=f"ot{pi}", bufs=1)
                        nc.vector.tensor_tensor(out=t1[:], in0=gq[:, :, 2], in1=gq[:, :, 0], op=OP.subtract)
                        nc.vector.tensor_tensor(out=t1[:], in0=frx[:], in1=t1[:], op=OP.mult)
                        nc.vector.tensor_tensor(out=t1[:], in0=gq[:, :, 0], in1=t1[:], op=OP.add)
                        nc.vector.tensor_tensor(out=t2[:], in0=gq[:, :, 3], in1=gq[:, :, 1], op=OP.subtract)
                        nc.vector.tensor_tensor(out=t2[:], in0=frx[:], in1=t2[:], op=OP.mult)
                        nc.vector.tensor_tensor(out=t2[:], in0=gq[:, :, 1], in1=t2[:], op=OP.add)
                        nc.vector.tensor_tensor(out=t2[:], in0=t2[:], in1=t1[:], op=OP.subtract)
                        nc.vector.tensor_tensor(out=t2[:], in0=fry[:], in1=t2[:], op=OP.mult)
                        nc.vector.tensor_tensor(out=ot[:], in0=t1[:], in1=t2[:], op=OP.add)
                        nc.sync.dma_start(out=outv[bass.ds(sv, 1), pi, :, :], in_=ot[:])

    nc.finalize()
    return nc


def _host_aux(translation, rotation, intrinsic):
    K = intrinsic.astype(np.float32)
    Kinv = np.linalg.inv(K).astype(np.float32)
    R = rotation.astype(np.float32)
    t = translation.astype(np.float32)
    nb = R.shape[0]
    temp = np.einsum('ij,bkj->bik', K, R).astype(np.float32)
    Wv = np.einsum('bij,bjk->bik', temp, -t).astype(np.float32)
    M = np.einsum('bij,jk->bik', temp, Kinv).astype(np.float32)
    W2 = np.einsum('ij,bjk->bik', K, t).astype(np.float32)
    M2 = np.einsum('bij,jk->bik', np.einsum('ij,bjk->bik', K, R), Kinv).astype(np.float32)

    x = np.arange(W, dtype=np.float32)
    y = np.arange(H, dtype=np.float32)
    ycols = y.reshape(NTILES, 128)                                   # [NTILES, 128]
    ycolsT = ycols.T                                                 # [128, NTILES]

    def rep_row(v):
        return np.repeat(v[:, None, :], 128, axis=1).astype(np.float32)

    aux = {}
    aux["rowA"] = rep_row(M[:, 0, 0][:, None] * x[None, :])
    aux["rowB"] = rep_row(M[:, 1, 0][:, None] * x[None, :])
    aux["rowC"] = rep_row(M[:, 2, 0][:, None] * x[None, :])
    aux["rowG"] = rep_row(M2[:, 2, 0][:, None] * x[None, :])
    # [nb, NTILES, 128, 1]
    aux["cA"] = (M[:, 0, 1][:, None, None] * ycols[None] + M[:, 0, 2][:, None, None]).astype(np.float32)[..., None]
    aux["cB"] = (M[:, 1, 1][:, None, None] * ycols[None] + M[:, 1, 2][:, None, None]).astype(np.float32)[..., None]
    aux["cC"] = (M[:, 2, 1][:, None, None] * ycols[None] + M[:, 2, 2][:, None, None]).astype(np.float32)[..., None]
    # phase A: cur rows r = 1+128b+p; prev rows r-1 = 128b+p  -> [nb, 128, NTILES]
    aux["gC"] = (M2[:, 2, 1][:, None, None] * (ycolsT[None] + 1.0) + M2[:, 2, 2][:, None, None]).astype(np.float32)
    aux["gP"] = (M2[:, 2, 1][:, None, None] * ycolsT[None] + M2[:, 2, 2][:, None, None]).astype(np.float32)
    ones = np.ones((nb, 128, 1), np.float32)
    aux["wx"] = Wv[:, 0, 0][:, None, None] * ones
    aux["wy"] = Wv[:, 1, 0][:, None, None] * ones
    aux["wz"] = Wv[:, 2, 0][:, None, None] * ones
    aux["w2"] = W2[:, 2, 0][:, None, None] * ones
    for k in aux:
        aux[k] = np.ascontiguousarray(aux[k].astype(np.float32))
    return aux


_NC_CACHE = {}


def kernel(depth_map_1, depth_map_2, translation, rotation, intrinsic):
    d1 = np.ascontiguousarray(np.asarray(depth_map_1, dtype=np.float32)[..., 0])
    d2 = np.ascontiguousarray(np.asarray(depth_map_2, dtype=np.float32)[..., 0])
    t = np.asarray(translation, dtype=np.float32)
    R = np.asarray(rotation, dtype=np.float32)
    K = np.asarray(intrinsic, dtype=np.float32)

    if "nc" not in _NC_CACHE:
        _NC_CACHE["nc"] = _build_bass()
    nc = _NC_CACHE["nc"]

    aux = _host_aux(t, R, K)

    in_maps = []
    for c in range(NCORES):
        sl = slice(c * BPC, (c + 1) * BPC)
        m = {"d1": d1[sl], "d2": d2[sl]}
        for k, v in aux.items():
            m[k] = v[sl]
        in_maps.append(m)

    res = bass_utils.run_bass_kernel_spmd(nc, in_maps, core_ids=list(range(NCORES)))
    outa = np.empty((B, H, W, 1), np.float32)
    for c in range(NCORES):
        outa[c * BPC:(c + 1) * BPC, :, :, 0] = res.results[c]["out"]
    return outa
